# revision 5
# baseline (speedup 1.0000x reference)
"""Trainium2 Bass kernel for nn_Attention_76089640616322.

Bahdanau-style attention:
  B, S, HE, DOUT = 32, 4096, 512, 512  (HD = 1024)
  energy = tanh(concat([context, broadcast(output)], -1) @ W1.T)   [B,S,HE]
  attn   = softmax(energy @ W2.T, axis=S)                           [B,1,S]
  mix    = attn @ context                                           [B,1,HE]
  out    = tanh(concat([mix, output], -1) @ Wout.T + bout)          [B,1,HD->HE]

Sharding: pure data parallel, batch dim across 8 cores (4 batches/core),
weights replicated.

v2 design (fp8 DoubleRow everywhere the PE streams wide):
- ctx is cast fp32->fp8e4 in the load DMA, laid out CT[k] = [p, ss, d] with
  s = k*512 + ss*128 + p so the ss dim doubles as the K-subtile dim for
  DoubleRow contractions over s (the mix).
- The broadcast `output` half of W1 folds into a per-batch per-partition
  bias on the tanh (ACT), so the energy matmul contracts only over d=512.
- W1cT is stored fp8 scaled by 64 (fp8e4 normals start at 2^-6; W1 ~ 0.02)
  as [p(d), ko, e] pair tiles; the 1/64 undoes inside the tanh's ACT scale.
- ctx is PE-transposed in fp8 (ISA requires psum element step 2, so the
  psum tiles are u16 lanes with the fp8 value in the low byte); one u16
  DVE copy per d-pair moves a half-block to SBUF at 2x.
- energyT = W1cT.T @ ctxT runs as DoubleRow fp8 matmuls: 0.5 cycles/row,
  2x the bf16 peak.
- logits: w2 (x64, fp8 pair column) stationary, tanhT tiles moving ->
  DoubleRow rows [1, 512] in psum; DVE stacks the 8 block rows into a
  partition-0 tile, one SBUF->SBUF DMA scatters to [8, 512], and 4 cheap
  [8,128] PE transposes columnize the logits for the softmax.
- softmax is unnormalized (exp with 1/64 ACT scale, fp32 row sums via the
  ACT accumulator); normalization folds in after the mix.
- mix = exp-weights (fp8, 16B-strided pairs) stationary, CT moving ->
  DoubleRow rows again; 16 matmuls cover s=4096 per batch.
- final: mix row -> bf16 columns (4 tiny transposes), 4 bf16 matmuls
  against WoutCT plus a K=1 bias-row matmul, tanh row, DMA out.
- Batch tails are deferred one batch so their serial chain overlaps the
  next batch's block pipeline.
"""

from contextlib import ExitStack

import numpy as np

import concourse.bass as bass
import concourse.tile as tile
from concourse import bacc, mybir
from concourse._compat import with_exitstack
from concourse.masks import make_identity

B, S, HE, DOUT = 32, 4096, 512, 512
HD = HE + DOUT
NCORES = 8
BC = B // NCORES  # batches per core

F32 = mybir.dt.float32
BF16 = mybir.dt.bfloat16
FP8 = mybir.dt.float8e4
U16 = mybir.dt.uint16
AF = mybir.ActivationFunctionType
DR = mybir.MatmulPerfMode.DoubleRow

NSBLK = 8           # s-blocks per batch (512 s each)
SBLK = S // NSBLK   # 512
NSS = SBLK // 128   # 4 subtiles of 128 s per block
NEC = HE // 128     # 4 e-chunks
W1SCALE = 64.0      # fp8 range lift for W1/W2 (~0.02 entries)


@with_exitstack
def attention_kernel(ctx: ExitStack, tc: tile.TileContext, out_ap, ins):
    nc = tc.nc

    ctx_ap = ins["context"]    # [BC, S, HE]
    outp_ap = ins["output"]    # [BC, 1, DOUT]
    w1_ap = ins["W1"]          # [HE, HD]
    w2_ap = ins["W2"]          # [1, HE]
    wout_ap = ins["Wout"]      # [HE, HD]
    bout_ap = ins["bout"]      # [HE]

    const = ctx.enter_context(tc.tile_pool(name="const", bufs=1))
    stage = ctx.enter_context(tc.tile_pool(name="stage", bufs=4))
    ct_pool = ctx.enter_context(tc.tile_pool(name="ct", bufs=16))
    xt_pool = ctx.enter_context(tc.tile_pool(name="xt", bufs=6))
    th_pool = ctx.enter_context(tc.tile_pool(name="th", bufs=4))
    small = ctx.enter_context(tc.tile_pool(name="small", bufs=2))
    lrs_pool = ctx.enter_context(tc.tile_pool(name="lrs", bufs=2))

    psum_tp = ctx.enter_context(tc.tile_pool(name="ptp", bufs=2, space="PSUM"))
    psum_en = ctx.enter_context(tc.tile_pool(name="pen", bufs=2, space="PSUM"))
    psum_lg = ctx.enter_context(tc.tile_pool(name="plg", bufs=2, space="PSUM"))
    psum_misc = ctx.enter_context(tc.tile_pool(name="pmisc", bufs=2, space="PSUM"))

    # ---- constants ----
    id128f = const.tile([128, 128], F32)
    make_identity(nc, id128f)
    id128e = const.tile([128, 128], FP8)
    nc.vector.tensor_copy(id128e, id128f)
    id128b = const.tile([128, 128], BF16)
    nc.vector.tensor_copy(id128b, id128f)
    ones1f = const.tile([1, 1], F32)
    nc.vector.memset(ones1f, 1.0)
    ones1b = const.tile([1, 1], BF16)
    nc.vector.memset(ones1b, 1.0)
    ones128 = const.tile([128, 1], F32)
    nc.vector.memset(ones128, 1.0)

    # ---- load weights (staged, fp32) ----
    w1_t = w1_ap.rearrange("(c p) d -> c p d", p=128)     # [4,128,1024]
    wout_t = wout_ap.rearrange("(c p) d -> c p d", p=128)
    w1sb = []
    for c in range(NEC):
        t1 = stage.tile([128, HD], F32, tag="stage")
        nc.sync.dma_start(out=t1, in_=w1_t[c])
        w1sb.append(t1)

    w2sb = const.tile([1, HE], F32)
    nc.sync.dma_start(out=w2sb, in_=w2_ap)
    boutsb = const.tile([1, HE], F32)
    nc.sync.dma_start(out=boutsb, in_=bout_ap.rearrange("(a d) -> a d", a=1))
    boutb = const.tile([1, HE], BF16)
    nc.vector.tensor_copy(boutb, boutsb)
    outp_rows = []
    for b in range(BC):
        t = const.tile([1, DOUT], F32, tag=f"outp_row{b}")
        nc.sync.dma_start(out=t, in_=outp_ap[b])
        outp_rows.append(t)

    # ---- transpose W1 -> w1T (8 tiles [d=128, e=512], fp32) ----
    w1T = []
    for dc in range(HD // 128):
        ps = psum_tp.tile([128, HE], F32, tag="tp")
        for ec in range(NEC):
            nc.tensor.transpose(
                ps[:, ec * 128:(ec + 1) * 128],
                w1sb[ec][:, dc * 128:(dc + 1) * 128],
                id128f,
            )
        dst = const.tile([128, HE], F32, tag=f"w1T{dc}")
        nc.vector.tensor_copy(dst, ps)
        w1T.append(dst)

    # fp8 DoubleRow stationary for the energy matmul: [p(d), ko, e] x2 dpairs
    w1p = []
    for j in range(2):
        t = const.tile([128, 2, HE], FP8, tag=f"w1p{j}")
        for ko in range(2):
            nc.scalar.activation(t[:, ko, :], w1T[2 * j + ko], AF.Copy,
                                 scale=W1SCALE)
        w1p.append(t)

    # ---- columnize W2 (fp8 x64 pairs, 16B k-pair stride), output cols ----
    ps = psum_misc.tile([128, NEC], F32, tag="misc")
    for ec in range(NEC):
        nc.tensor.transpose(
            ps[:, ec:ec + 1], w2sb[:, ec * 128:(ec + 1) * 128], ones1f
        )
    w2col = const.tile([128, NEC], F32)
    nc.vector.tensor_copy(w2col, ps)
    w2p = []
    for j in range(2):
        t = const.tile([128, 2, 16], FP8, tag=f"w2p{j}")
        for ko in range(2):
            nc.vector.tensor_scalar_mul(
                t[:, ko, 0:1], w2col[:, 2 * j + ko:2 * j + ko + 1], W1SCALE
            )
        w2p.append(t)

    # output_b columns: outpcol[:, b*4+dc] = output[b, dc*128 + p]
    ps = psum_misc.tile([128, BC * 4], F32, tag="misc")
    for b in range(BC):
        for dc in range(4):
            nc.tensor.transpose(
                ps[:, b * 4 + dc: b * 4 + dc + 1],
                outp_rows[b][:, dc * 128:(dc + 1) * 128],
                ones1f,
            )
    outpcol = const.tile([128, BC * 4], F32)
    nc.vector.tensor_copy(outpcol, ps)
    outpcolb = const.tile([128, BC * 4], BF16)
    nc.vector.tensor_copy(outpcolb, outpcol)

    # ---- per-batch tanh offsets: off[b] = W1[:, HE:] @ output_b ----
    ps = psum_misc.tile([128, BC * NEC], F32, tag="misc")
    for b in range(BC):
        for ec in range(NEC):
            for dco in range(4):
                nc.tensor.matmul(
                    ps[:, b * NEC + ec: b * NEC + ec + 1],
                    lhsT=w1T[4 + dco][:, ec * 128:(ec + 1) * 128],
                    rhs=outpcol[:, b * 4 + dco: b * 4 + dco + 1],
                    start=(dco == 0),
                    stop=(dco == 3),
                )
    offsb = const.tile([128, BC * NEC], F32)
    nc.vector.tensor_copy(offsb, ps)

    # ---- Wout setup (deferred into batch 0's shadow, like baseline) ----
    woutTb = []   # bf16 [d=128, e=512] for dc 0..3 (mix half)
    browb = []    # bf16 [1, 512] per batch: bout + Wout[:, HE:] @ output_b

    def emit_wout_setup():
        woutsb = []
        for c in range(NEC):
            t2 = stage.tile([128, HD], F32, tag="stage")
            nc.sync.dma_start(out=t2, in_=wout_t[c])
            woutsb.append(t2)
        woutT_hi = []
        for dc in range(HD // 128):
            ps = psum_tp.tile([128, HE], F32, tag="tp")
            for ec in range(NEC):
                nc.tensor.transpose(
                    ps[:, ec * 128:(ec + 1) * 128],
                    woutsb[ec][:, dc * 128:(dc + 1) * 128],
                    id128f,
                )
            if dc < 4:
                dstb = const.tile([128, HE], BF16, tag=f"woutTb{dc}")
                nc.vector.tensor_copy(dstb, ps)
                woutTb.append(dstb)
            else:
                dstb = const.tile([128, HE], BF16, tag=f"woutThi{dc}")
                nc.vector.tensor_copy(dstb, ps)
                woutT_hi.append(dstb)
        for b in range(BC):
            pb = psum_misc.tile([1, HE], F32, tag="misc")
            for dco in range(4):
                nc.tensor.matmul(
                    pb,
                    lhsT=outpcolb[:, b * 4 + dco: b * 4 + dco + 1],
                    rhs=woutT_hi[dco],
                    start=(dco == 0),
                    stop=False,
                )
            nc.tensor.matmul(pb, lhsT=ones1b, rhs=boutb, start=False, stop=True)
            br = const.tile([1, HE], BF16, tag=f"brow{b}")
            nc.vector.tensor_copy(br, pb)
            browb.append(br)

    # ---- main loop over batches (tails deferred one batch for overlap) ----
    def emit_blocks(b):
        ctx_b = ctx_ap[b].rearrange("(k ss p) d -> k p ss d", ss=NSS, p=128)
        ct_tiles = []
        lrs = lrs_pool.tile([1, NSBLK, SBLK], BF16, tag="lrs")
        lg_pending = [None]

        def logit_mms(kk, th):
            pl = psum_lg.tile([1, SBLK], F32, tag="lg")
            for j in range(2):
                nc.tensor.matmul(
                    pl,
                    lhsT=w2p[j][:, :, 0:1],
                    rhs=th[:, 2 * j:2 * j + 2, :],
                    start=(j == 0),
                    stop=(j == 1),
                    perf_mode=DR,
                )
            nc.vector.tensor_copy(lrs[:, kk, :], pl)

        def load_and_transpose(k):
            # one s-block, cast fp32 -> fp8e4 in the DMA
            ct = ct_pool.tile([128, NSS, HE], FP8, tag="ct")
            nc.gpsimd.dma_start(out=ct, in_=ctx_b[k])

            # fp8 PE transposes into u16 psum lanes (value in low byte),
            # one tile per d-pair; single u16 DVE copyback each (2x mode)
            xt = []
            for h in range(2):
                tp = psum_tp.tile([128, 2, SBLK], U16, tag="tp")
                tp8 = tp.bitcast(FP8).rearrange(
                    "p c (s two) -> p c s two", two=2
                )
                for c in range(2):
                    dc = 2 * h + c
                    for ss in range(NSS):
                        nc.tensor.transpose(
                            tp8[:, c, ss * 128:(ss + 1) * 128, 0],
                            ct[:, ss, dc * 128:(dc + 1) * 128],
                            id128e,
                        )
                sb = xt_pool.tile([128, 2, SBLK], U16, tag="xt")
                nc.vector.tensor_copy(sb, tp)
                xt.append(
                    sb.bitcast(FP8).rearrange("p c (s two) -> p c s two", two=2)
                )
            return ct, xt

        cur = load_and_transpose(0)
        for k in range(NSBLK):
            ct, xt = cur
            ct_tiles.append(ct)
            if k + 1 < NSBLK:
                cur = load_and_transpose(k + 1)

            # energyT[ec] = sum_j W1P[j].T @ ctxT[j]  (DoubleRow fp8)
            th = th_pool.tile([128, NEC, SBLK], FP8, tag="th")
            for ec in range(NEC):
                pe = psum_en.tile([128, SBLK], F32, tag="en")
                for j in range(2):
                    nc.tensor.matmul(
                        pe,
                        lhsT=w1p[j][:, :, ec * 128:(ec + 1) * 128],
                        rhs=xt[j][:, :, :, 0],
                        start=(j == 0),
                        stop=(j == 1),
                        perf_mode=DR,
                    )
                nc.scalar.activation(
                    th[:, ec, :], pe, AF.Tanh,
                    bias=offsb[:, b * NEC + ec: b * NEC + ec + 1],
                    scale=1.0 / W1SCALE,
                )

            # logits for the PREVIOUS block so the PE's static order never
            # waits on a tanh ACT has only just been issued
            if lg_pending[0] is not None:
                logit_mms(*lg_pending[0])
            lg_pending[0] = (k, th)
        logit_mms(*lg_pending[0])
        return ct_tiles, lrs

    def emit_tail(b, ct_tiles, lrs):
        # scatter logit rows to 8 partitions (SBUF->SBUF DMA)
        lrt = small.tile([NSBLK, SBLK], BF16, tag="lrt")
        nc.sync.dma_start(out=lrt, in_=lrs.rearrange("a k f -> a (k f)"))

        # bridge transposes -> logit columns [g, (c, k)] in psum (bf16)
        brt = psum_misc.tile([128, NSS, NSBLK], BF16, tag="misc")
        for c in range(NSS):
            nc.tensor.transpose(
                brt[:, c, :],
                lrt[:, c * 128:(c + 1) * 128],
                id128b[:NSBLK, :NSBLK],
            )

        # exp (unnormalized softmax): cols j = k*4 + c, 16B-strided fp8
        exf = small.tile([128, NSBLK * NSS, 16], FP8, tag="exf")
        rowsum = small.tile([128, 1], F32, tag="rowsum")
        nc.scalar.activation(
            exf[:, :, 0].rearrange("p (k c) -> p k c", c=NSS),
            brt.rearrange("p c k -> p k c"),
            AF.Exp,
            scale=1.0 / W1SCALE,
            accum_out=rowsum,
        )

        pd = psum_misc.tile([1, 1], F32, tag="misc")
        nc.tensor.matmul(pd, lhsT=rowsum, rhs=ones128)
        inv = small.tile([1, 1], F32, tag="inv")
        nc.vector.reciprocal(inv, pd)

        # mix row: exp pairs stationary, CT moving (DoubleRow over s)
        pm = psum_misc.tile([1, HE], F32, tag="misc")
        for u in range(S // 256):
            k, c0 = u // 2, (u % 2) * 2
            nc.tensor.matmul(
                pm,
                lhsT=exf[:, 2 * u:2 * u + 2, 0:1],
                rhs=ct_tiles[k][:, c0:c0 + 2, :],
                start=(u == 0),
                stop=(u == S // 256 - 1),
                perf_mode=DR,
            )
        mrow = small.tile([1, HE], BF16, tag="mrow")
        nc.vector.tensor_scalar_mul(mrow, pm, inv)

        # mix row -> bf16 columns (4-byte-aligned psum slots)
        pmc = psum_misc.tile([128, 4, 2], BF16, tag="misc")
        for dc in range(4):
            nc.tensor.transpose(
                pmc[:, dc, 0:1], mrow[:, dc * 128:(dc + 1) * 128], ones1b
            )
        mc = small.tile([128, 4], BF16, tag="mc")
        nc.vector.tensor_copy(mc, pmc[:, :, 0])

        # final: out_row = tanh(sum_dc WoutCT[dc].T-col @ ... + brow)
        pf = psum_misc.tile([1, HE], F32, tag="misc")
        for dc in range(4):
            nc.tensor.matmul(
                pf, lhsT=mc[:, dc:dc + 1], rhs=woutTb[dc],
                start=(dc == 0), stop=False,
            )
        nc.tensor.matmul(pf, lhsT=ones1b, rhs=browb[b], start=False, stop=True)
        orow = small.tile([1, HE], F32, tag="orow")
        nc.scalar.activation(orow, pf, AF.Tanh)
        nc.sync.dma_start(out=out_ap[b], in_=orow)

    pending = None
    for b in range(BC):
        state = emit_blocks(b)
        if b == 0:
            emit_wout_setup()
        if pending is not None:
            emit_tail(pending[0], *pending[1])
        pending = (b, state)
    emit_tail(pending[0], *pending[1])


INPUT_SPECS = {
    "output": ((BC, 1, DOUT), F32),
    "context": ((BC, S, HE), F32),
    "W1": ((HE, HD), F32),
    "W2": ((1, HE), F32),
    "Wout": ((HE, HD), F32),
    "bout": ((HE,), F32),
}

_CACHE = {}


def build_nc():
    if "nc" in _CACHE:
        return _CACHE["nc"]
    nc = bacc.Bacc("TRN2", target_bir_lowering=False, debug=False,
                   num_devices=NCORES)
    ins = {
        name: nc.dram_tensor(name, list(shape), dt, kind="ExternalInput").ap()
        for name, (shape, dt) in INPUT_SPECS.items()
    }
    out = nc.dram_tensor("out", [BC, 1, HE], F32, kind="ExternalOutput").ap()
    with tile.TileContext(nc) as tc:
        attention_kernel(tc, out, ins)
    nc.compile()
    _CACHE["nc"] = nc
    return nc


def make_in_maps(output, context, W1, W2, Wout, bout):
    maps = []
    for i in range(NCORES):
        sl = slice(i * BC, (i + 1) * BC)
        maps.append({
            "output": np.ascontiguousarray(output[sl], dtype=np.float32),
            "context": np.ascontiguousarray(context[sl], dtype=np.float32),
            "W1": np.ascontiguousarray(W1, dtype=np.float32),
            "W2": np.ascontiguousarray(W2, dtype=np.float32),
            "Wout": np.ascontiguousarray(Wout, dtype=np.float32),
            "bout": np.ascontiguousarray(bout, dtype=np.float32),
        })
    return maps


def run(inputs, trace=False):
    from concourse.bass_utils import run_bass_kernel_spmd

    nc = build_nc()
    in_maps = make_in_maps(**inputs)
    res = run_bass_kernel_spmd(nc, in_maps, list(range(NCORES)), trace=trace)
    out = np.concatenate([res.results[i]["out"] for i in range(NCORES)], axis=0)
    return out, res


def kernel(output, context, W1, W2, Wout, bout):
    out, _ = run(dict(output=output, context=context, W1=W1, W2=W2,
                      Wout=Wout, bout=bout))
    return out


# revision 29
# speedup vs baseline: 1.6109x; 1.6109x over previous
"""Trainium2 Bass kernel for nn_Attention_76089640616322.

Bahdanau-style attention:
  B, S, HE, DOUT = 32, 4096, 512, 512  (HD = 1024)
  energy = tanh(concat([context, broadcast(output)], -1) @ W1.T)   [B,S,HE]
  attn   = softmax(energy @ W2.T, axis=S)                           [B,1,S]
  mix    = attn @ context                                           [B,1,HE]
  out    = tanh(concat([mix, output], -1) @ Wout.T + bout)          [B,1,HD->HE]

Sharding: pure data parallel, batch dim across 8 cores (4 batches/core),
weights replicated.

v2 design (fp8 DoubleRow everywhere the PE streams wide):
- ctx is cast fp32->fp8e4 in the load DMA, laid out CT[k] = [p, ss, d] with
  s = k*512 + ss*128 + p so the ss dim doubles as the K-subtile dim for
  DoubleRow contractions over s (the mix).
- The broadcast `output` half of W1 folds into a per-batch per-partition
  bias on the tanh (ACT), so the energy matmul contracts only over d=512.
- W1cT is stored fp8 scaled by 64 (fp8e4 normals start at 2^-6; W1 ~ 0.02)
  as [p(d), ko, e] pair tiles; the 1/64 undoes inside the tanh's ACT scale.
- ctx is PE-transposed in fp8 (ISA requires psum element step 2, so the
  psum tiles are u16 lanes with the fp8 value in the low byte); one u16
  DVE copy per d-pair moves a half-block to SBUF at 2x.
- energyT = W1cT.T @ ctxT runs as DoubleRow fp8 matmuls: 0.5 cycles/row,
  2x the bf16 peak.
- logits: w2 (x64, fp8 pair column) stationary, tanhT tiles moving ->
  DoubleRow rows [1, 512] in psum; DVE stacks the 8 block rows into a
  partition-0 tile, one SBUF->SBUF DMA scatters to [8, 512], and 4 cheap
  [8,128] PE transposes columnize the logits for the softmax.
- softmax is unnormalized (exp with 1/64 ACT scale, fp32 row sums via the
  ACT accumulator); normalization folds in after the mix.
- mix = exp-weights (fp8, 16B-strided pairs) stationary, CT moving ->
  DoubleRow rows again; 16 matmuls cover s=4096 per batch.
- final: mix row -> bf16 columns (4 tiny transposes), 4 bf16 matmuls
  against WoutCT plus a K=1 bias-row matmul, tanh row, DMA out.
- Batch tails are deferred one batch so their serial chain overlaps the
  next batch's block pipeline.
"""

from contextlib import ExitStack

import numpy as np

import concourse.bass as bass
import concourse.tile as tile
from concourse import bacc, mybir
from concourse._compat import with_exitstack
from concourse.masks import make_identity

B, S, HE, DOUT = 32, 4096, 512, 512
HD = HE + DOUT
NCORES = 8
BC = B // NCORES  # batches per core

F32 = mybir.dt.float32
BF16 = mybir.dt.bfloat16
FP8 = mybir.dt.float8e4
U16 = mybir.dt.uint16
AF = mybir.ActivationFunctionType
DR = mybir.MatmulPerfMode.DoubleRow

NSBLK = 8           # s-blocks per batch (512 s each)
SBLK = S // NSBLK   # 512
NSS = SBLK // 128   # 4 subtiles of 128 s per block
NEC = HE // 128     # 4 e-chunks
W1SCALE = 64.0      # fp8 range lift for W1/W2 (~0.02 entries)


@with_exitstack
def attention_kernel(ctx: ExitStack, tc: tile.TileContext, out_ap, ins):
    nc = tc.nc

    ctx_ap = ins["context"]    # [BC, S, HE]
    outp_ap = ins["output"]    # [BC, 1, DOUT]
    w1_ap = ins["W1"]          # [HE, HD]
    w2_ap = ins["W2"]          # [1, HE]
    wout_ap = ins["Wout"]      # [HE, HD]
    bout_ap = ins["bout"]      # [HE]

    const = ctx.enter_context(tc.tile_pool(name="const", bufs=1))
    stage = ctx.enter_context(tc.tile_pool(name="stage", bufs=4))
    ct_pool = ctx.enter_context(tc.tile_pool(name="ct", bufs=8))

    # issue the first context-pair cast DMA before anything else queues on
    # the gpsimd (SWDGE) engine, so the transfer overlaps all weight setup
    ctx_b0 = ctx_ap[0].rearrange("(q k ss p) d -> q p k ss d",
                                 k=2, ss=NSS, p=128)
    ct_first = ct_pool.tile([128, 2, NSS, HE], FP8, tag="ct")
    nc.gpsimd.dma_start(out=ct_first, in_=ctx_b0[0])
    xt_pool = ctx.enter_context(tc.tile_pool(name="xt", bufs=6))
    th_pool = ctx.enter_context(tc.tile_pool(name="th", bufs=4))
    small = ctx.enter_context(tc.tile_pool(name="small", bufs=2))
    lrs_pool = ctx.enter_context(tc.tile_pool(name="lrs", bufs=2))

    psum_tp = ctx.enter_context(tc.tile_pool(name="ptp", bufs=2, space="PSUM"))
    psum_en = ctx.enter_context(tc.tile_pool(name="pen", bufs=2, space="PSUM"))
    psum_lg = ctx.enter_context(tc.tile_pool(name="plg", bufs=1, space="PSUM"))
    psum_misc = ctx.enter_context(tc.tile_pool(name="pmisc", bufs=1, space="PSUM"))

    # ---- constants ----
    id128f = const.tile([128, 128], F32)
    make_identity(nc, id128f)
    id128e = const.tile([128, 128], FP8)
    nc.vector.tensor_copy(id128e, id128f)
    id128b = const.tile([128, 128], BF16)
    nc.vector.tensor_copy(id128b, id128f)
    ones1f = const.tile([1, 1], F32)
    nc.vector.memset(ones1f, 1.0)
    ones1b = const.tile([1, 1], BF16)
    nc.vector.memset(ones1b, 1.0)
    ones128 = const.tile([128, 1], F32)
    nc.vector.memset(ones128, 1.0)

    # ---- load weights (staged, fp32) ----
    w1_t = w1_ap.rearrange("(c p) d -> c p d", p=128)     # [4,128,1024]
    wout_t = wout_ap.rearrange("(c p) d -> c p d", p=128)
    w1sb = []
    for c in range(NEC):
        t1 = stage.tile([128, HD], F32, tag="stage")
        nc.sync.dma_start(out=t1, in_=w1_t[c])
        w1sb.append(t1)

    w2sb = const.tile([1, HE], F32)
    nc.sync.dma_start(out=w2sb, in_=w2_ap)
    boutsb = const.tile([1, HE], F32)
    nc.sync.dma_start(out=boutsb, in_=bout_ap.rearrange("(a d) -> a d", a=1))
    boutb = const.tile([1, HE], BF16)
    nc.vector.tensor_copy(boutb, boutsb)
    outp_rows = []
    for b in range(BC):
        t = const.tile([1, DOUT], F32, tag=f"outp_row{b}")
        nc.sync.dma_start(out=t, in_=outp_ap[b])
        outp_rows.append(t)

    # ---- transpose W1 -> w1T (8 tiles [d=128, e=512], fp32) ----
    # context half (dc 0-3) first: it gates the first energy matmuls via
    # W1P; the output half (dc 4-7) only feeds the tanh-bias offsets
    w1T = [None] * (HD // 128)
    w1p = [None, None]
    for dc in list(range(4)) + list(range(4, HD // 128)):
        ps = psum_tp.tile([128, HE], F32, tag="tp")
        for ec in range(NEC):
            nc.tensor.transpose(
                ps[:, ec * 128:(ec + 1) * 128],
                w1sb[ec][:, dc * 128:(dc + 1) * 128],
                id128f,
            )
        dst = const.tile([128, HE], F32, tag=f"w1T{dc}")
        nc.vector.tensor_copy(dst, ps)
        w1T[dc] = dst
        if dc in (1, 3):
            # fp8 DoubleRow stationary [p(d), ko, e] as soon as a pair lands
            j = dc // 2
            t = const.tile([128, 2, HE], FP8, tag=f"w1p{j}")
            for ko in range(2):
                nc.scalar.activation(t[:, ko, :], w1T[2 * j + ko], AF.Copy,
                                     scale=W1SCALE)
            w1p[j] = t

    # ---- columnize W2 (fp8 x64 pairs, 16B k-pair stride), output cols ----
    ps = psum_misc.tile([128, NEC], F32, tag="misc")
    for ec in range(NEC):
        nc.tensor.transpose(
            ps[:, ec:ec + 1], w2sb[:, ec * 128:(ec + 1) * 128], ones1f
        )
    w2col = const.tile([128, NEC], F32)
    nc.vector.tensor_copy(w2col, ps)
    w2p = []
    for j in range(2):
        t = const.tile([128, 2, 16], FP8, tag=f"w2p{j}")
        for ko in range(2):
            nc.vector.tensor_scalar_mul(
                t[:, ko, 0:1], w2col[:, 2 * j + ko:2 * j + ko + 1], W1SCALE
            )
        w2p.append(t)

    # output_b columns: outpcol[:, b*4+dc] = output[b, dc*128 + p]
    ps = psum_misc.tile([128, BC * 4], F32, tag="misc")
    for b in range(BC):
        for dc in range(4):
            nc.tensor.transpose(
                ps[:, b * 4 + dc: b * 4 + dc + 1],
                outp_rows[b][:, dc * 128:(dc + 1) * 128],
                ones1f,
            )
    outpcol = const.tile([128, BC * 4], F32)
    nc.vector.tensor_copy(outpcol, ps)
    outpcolb = const.tile([128, BC * 4], BF16)
    nc.vector.tensor_copy(outpcolb, outpcol)

    # ---- per-batch tanh offsets: off[b] = W1[:, HE:] @ output_b ----
    ps = psum_misc.tile([128, BC * NEC], F32, tag="misc")
    for b in range(BC):
        for ec in range(NEC):
            for dco in range(4):
                nc.tensor.matmul(
                    ps[:, b * NEC + ec: b * NEC + ec + 1],
                    lhsT=w1T[4 + dco][:, ec * 128:(ec + 1) * 128],
                    rhs=outpcol[:, b * 4 + dco: b * 4 + dco + 1],
                    start=(dco == 0),
                    stop=(dco == 3),
                )
    offsb = const.tile([128, BC * NEC], F32)
    nc.vector.tensor_copy(offsb, ps)

    # ---- Wout setup, spread in small chunks across batch 0's blocks ----
    woutTb = []   # bf16 [d=128, e=512] for dc 0..3 (mix half)
    browb = []    # bf16 [1, 512] per batch: bout + Wout[:, HE:] @ output_b
    woutsb = []
    woutT_hi = []

    def wout_setup_steps():
        def load():
            for c in range(NEC):
                t2 = stage.tile([128, HD], F32, tag="stage")
                nc.sync.dma_start(out=t2, in_=wout_t[c])
                woutsb.append(t2)
        yield load

        def tp_step(dc):
            ps = psum_tp.tile([128, HE], F32, tag="tp")
            for ec in range(NEC):
                nc.tensor.transpose(
                    ps[:, ec * 128:(ec + 1) * 128],
                    woutsb[ec][:, dc * 128:(dc + 1) * 128],
                    id128f,
                )
            tag = f"woutTb{dc}" if dc < 4 else f"woutThi{dc}"
            dstb = const.tile([128, HE], BF16, tag=tag)
            nc.vector.tensor_copy(dstb, ps)
            (woutTb if dc < 4 else woutT_hi).append(dstb)
        for dc in range(HD // 128):
            yield lambda dc=dc: tp_step(dc)

        def brow_step(b):
            pb = psum_misc.tile([1, HE], F32, tag="misc")
            for dco in range(4):
                nc.tensor.matmul(
                    pb,
                    lhsT=outpcolb[:, b * 4 + dco: b * 4 + dco + 1],
                    rhs=woutT_hi[dco],
                    start=(dco == 0),
                    stop=False,
                )
            nc.tensor.matmul(pb, lhsT=ones1b, rhs=boutb, start=False, stop=True)
            br = const.tile([1, HE], BF16, tag=f"brow{b}")
            nc.vector.tensor_copy(br, pb)
            browb.append(br)
        for b in range(BC):
            yield lambda b=b: brow_step(b)

    # ---- main loop over batches (tails deferred one batch for overlap) ----
    def emit_blocks(b, setup_steps=None, preloaded=None):
        ctx_b = ctx_ap[b].rearrange("(q k ss p) d -> q p k ss d",
                                    k=2, ss=NSS, p=128)
        ct_tiles = []
        lrs = lrs_pool.tile([1, NSBLK, SBLK], BF16, tag="lrs")
        lg_pending = [None]

        def logit_mms(pr, th, i):
            pl = psum_lg.tile([1, SBLK], F32, tag="lg")
            for j in range(2):
                nc.tensor.matmul(
                    pl,
                    lhsT=w2p[j][:, :, 0:1],
                    rhs=th[:, 2 * j:2 * j + 2, i, :],
                    start=(j == 0),
                    stop=(j == 1),
                    perf_mode=DR,
                )
            nc.vector.tensor_copy(lrs[:, 2 * pr + i, :], pl)

        def load_pair(q):
            # one s-block PAIR in a single cast DMA (fp32 -> fp8e4)
            ct = ct_pool.tile([128, 2, NSS, HE], FP8, tag="ct")
            nc.gpsimd.dma_start(out=ct, in_=ctx_b[q])
            return ct

        def transpose_block(ct, i):
            # fp8 PE transposes into u16 psum lanes (value in low byte),
            # one tile per d-pair; single u16 DVE copyback each (2x mode)
            xt = []
            for h in range(2):
                tp = psum_tp.tile([128, 2, SBLK], U16, tag="tp")
                tp8 = tp.bitcast(FP8).rearrange(
                    "p c (s two) -> p c s two", two=2
                )
                for c in range(2):
                    dc = 2 * h + c
                    for ss in range(NSS):
                        nc.tensor.transpose(
                            tp8[:, c, ss * 128:(ss + 1) * 128, 0],
                            ct[:, i, ss, dc * 128:(dc + 1) * 128],
                            id128e,
                        )
                sb = xt_pool.tile([128, 2, SBLK], U16, tag="xt")
                nc.vector.tensor_copy(sb, tp)
                xt.append(
                    sb.bitcast(FP8).rearrange("p c (s two) -> p c s two", two=2)
                )
            return xt

        # process blocks in pairs: one [128, 2, 512] energy psum (2 banks)
        # per ec so each tanh ACT covers 1024 elements (same ec, same bias).
        # Next pair's transposes sit between ec01 and ec23 so the PE never
        # waits for a tanh to free an energy psum bank.
        def energy_half(xts, th, ecs):
            for ec in ecs:
                pe = psum_en.tile([128, 2, SBLK], F32, tag="en")
                for i, xt in enumerate(xts):
                    for j in range(2):
                        nc.tensor.matmul(
                            pe[:, i, :],
                            lhsT=w1p[j][:, :, ec * 128:(ec + 1) * 128],
                            rhs=xt[j][:, :, :, 0],
                            start=(j == 0),
                            stop=(j == 1),
                            perf_mode=DR,
                        )
                nc.scalar.activation(
                    th[:, ec, :, :], pe, AF.Tanh,
                    bias=offsb[:, b * NEC + ec: b * NEC + ec + 1],
                    scale=1.0 / W1SCALE,
                )

        npair = NSBLK // 2
        cts = [preloaded if preloaded is not None else load_pair(0),
               load_pair(1)]
        xt_cur = [transpose_block(cts[0], 0), transpose_block(cts[0], 1)]
        for pr in range(npair):
            xt0, xt1 = xt_cur
            ct_tiles.append(cts[pr])
            if pr + 2 < npair:
                cts.append(load_pair(pr + 2))

            th = th_pool.tile([128, NEC, 2, SBLK], FP8, tag="th")
            energy_half((xt0, xt1), th, (0, 1))
            # logits for the PREVIOUS pair, one block here and one at pair
            # end: a half-pair apart, so the single lg psum bank recycles
            # without ever stalling the PE behind the DVE row copy
            if lg_pending[0] is not None:
                logit_mms(*lg_pending[0], 0)
            if pr + 1 < npair:
                xt_cur = [transpose_block(cts[pr + 1], 0),
                          transpose_block(cts[pr + 1], 1)]
            energy_half((xt0, xt1), th, (2, 3))
            if lg_pending[0] is not None:
                logit_mms(*lg_pending[0], 1)
            lg_pending[0] = (pr, th)
            if setup_steps is not None:
                for _ in range(4):
                    step = next(setup_steps, None)
                    if step is not None:
                        step()
        logit_mms(*lg_pending[0], 0)
        logit_mms(*lg_pending[0], 1)
        return ct_tiles, lrs

    def emit_tail(b, ct_tiles, lrs):
        # scatter logit rows to 8 partitions (SBUF->SBUF DMA)
        lrt = small.tile([NSBLK, SBLK], BF16, tag="lrt")
        nc.sync.dma_start(out=lrt, in_=lrs.rearrange("a k f -> a (k f)"))

        # bridge transposes -> logit columns [g, (c, k)] in psum (bf16)
        brt = psum_misc.tile([128, NSS, NSBLK], BF16, tag="misc")
        for c in range(NSS):
            nc.tensor.transpose(
                brt[:, c, :],
                lrt[:, c * 128:(c + 1) * 128],
                id128b[:NSBLK, :NSBLK],
            )

        # exp (unnormalized softmax): cols j = k*4 + c, 16B-strided fp8
        exf = small.tile([128, NSBLK * NSS, 16], FP8, tag="exf")
        rowsum = small.tile([128, 1], F32, tag="rowsum")
        nc.scalar.activation(
            exf[:, :, 0].rearrange("p (k c) -> p k c", c=NSS),
            brt.rearrange("p c k -> p k c"),
            AF.Exp,
            scale=1.0 / W1SCALE,
            accum_out=rowsum,
        )

        pd = psum_misc.tile([1, 1], F32, tag="misc")
        nc.tensor.matmul(pd, lhsT=rowsum, rhs=ones128)
        inv = small.tile([1, 1], F32, tag="inv")
        nc.vector.reciprocal(inv, pd)

        # mix row: exp pairs stationary, CT moving (DoubleRow over s)
        pm = psum_misc.tile([1, HE], F32, tag="misc")
        for u in range(S // 256):
            k, c0 = u // 2, (u % 2) * 2
            nc.tensor.matmul(
                pm,
                lhsT=exf[:, 2 * u:2 * u + 2, 0:1],
                rhs=ct_tiles[k // 2][:, k % 2, c0:c0 + 2, :],
                start=(u == 0),
                stop=(u == S // 256 - 1),
                perf_mode=DR,
            )
        mrow = small.tile([1, HE], BF16, tag="mrow")
        nc.vector.tensor_scalar_mul(mrow, pm, inv)

        # mix row -> bf16 columns (4-byte-aligned psum slots)
        pmc = psum_misc.tile([128, 4, 2], BF16, tag="misc")
        for dc in range(4):
            nc.tensor.transpose(
                pmc[:, dc, 0:1], mrow[:, dc * 128:(dc + 1) * 128], ones1b
            )
        mc = small.tile([128, 4], BF16, tag="mc")
        nc.vector.tensor_copy(mc, pmc[:, :, 0])

        # final: out_row = tanh(sum_dc WoutCT[dc].T-col @ ... + brow)
        pf = psum_misc.tile([1, HE], F32, tag="misc")
        for dc in range(4):
            nc.tensor.matmul(
                pf, lhsT=mc[:, dc:dc + 1], rhs=woutTb[dc],
                start=(dc == 0), stop=False,
            )
        nc.tensor.matmul(pf, lhsT=ones1b, rhs=browb[b], start=False, stop=True)
        orow = small.tile([1, HE], F32, tag="orow")
        nc.scalar.activation(orow, pf, AF.Tanh)
        nc.sync.dma_start(out=out_ap[b], in_=orow)

    pending = None
    steps = wout_setup_steps()
    for b in range(BC):
        state = emit_blocks(b, setup_steps=steps if b == 0 else None,
                            preloaded=ct_first if b == 0 else None)
        if pending is not None:
            emit_tail(pending[0], *pending[1])
        pending = (b, state)
    emit_tail(pending[0], *pending[1])


INPUT_SPECS = {
    "output": ((BC, 1, DOUT), F32),
    "context": ((BC, S, HE), F32),
    "W1": ((HE, HD), F32),
    "W2": ((1, HE), F32),
    "Wout": ((HE, HD), F32),
    "bout": ((HE,), F32),
}

_CACHE = {}


def build_nc():
    if "nc" in _CACHE:
        return _CACHE["nc"]
    nc = bacc.Bacc("TRN2", target_bir_lowering=False, debug=False,
                   num_devices=NCORES)
    ins = {
        name: nc.dram_tensor(name, list(shape), dt, kind="ExternalInput").ap()
        for name, (shape, dt) in INPUT_SPECS.items()
    }
    out = nc.dram_tensor("out", [BC, 1, HE], F32, kind="ExternalOutput").ap()
    with tile.TileContext(nc) as tc:
        attention_kernel(tc, out, ins)
    nc.compile()
    _CACHE["nc"] = nc
    return nc


def make_in_maps(output, context, W1, W2, Wout, bout):
    maps = []
    for i in range(NCORES):
        sl = slice(i * BC, (i + 1) * BC)
        maps.append({
            "output": np.ascontiguousarray(output[sl], dtype=np.float32),
            "context": np.ascontiguousarray(context[sl], dtype=np.float32),
            "W1": np.ascontiguousarray(W1, dtype=np.float32),
            "W2": np.ascontiguousarray(W2, dtype=np.float32),
            "Wout": np.ascontiguousarray(Wout, dtype=np.float32),
            "bout": np.ascontiguousarray(bout, dtype=np.float32),
        })
    return maps


def run(inputs, trace=False):
    from concourse.bass_utils import run_bass_kernel_spmd

    nc = build_nc()
    in_maps = make_in_maps(**inputs)
    res = run_bass_kernel_spmd(nc, in_maps, list(range(NCORES)), trace=trace)
    out = np.concatenate([res.results[i]["out"] for i in range(NCORES)], axis=0)
    return out, res


def kernel(output, context, W1, W2, Wout, bout):
    out, _ = run(dict(output=output, context=context, W1=W1, W2=W2,
                      Wout=Wout, bout=bout))
    return out
